# revision 9
# baseline (speedup 1.0000x reference)
"""Trainium2 Bass kernel for the MultiHeadAttention (transformer-XL style) problem.

Data-parallel over batch: 8 cores, 2 output batches each. The reference's raw
row-major reshapes mean k = kv[:16] draws from underlying batches 0-7 and
v = kv[16:] from batches 8-15, so core c needs kv projections of underlying
batches c (K source) and 8+c (V source) -- still fully local per core.

Everything on-chip is computed in transposed orientation (contraction dim on
partitions): score^T[j,i] tiles accumulate AC^T (matmul) + shifted-BD^T
(HBM roundtrip with a negative-step strided read) + band mask; exp on ScalarE;
softmax denominators via ones-column matmuls (partition sums); normalization
deferred past the V matmul via a K=1 broadcast matmul.
"""

import sys

for _p in ("/opt/trn_rl_repo",):
    if _p not in sys.path:
        sys.path.insert(0, _p)

import numpy as np

import concourse.bass as bass
import concourse.mybir as mybir
import concourse.tile as tile
from concourse import bacc
from concourse.bass_utils import run_bass_kernel_spmd

F32 = mybir.dt.float32
BF16 = mybir.dt.bfloat16

B, SEG, MEM_L, MD, H, D = 16, 512, 512, 128, 8, 128
TOTAL = SEG + MEM_L  # 1024
NCORES = 8
INV_SQRT_D = 1.0 / float(np.sqrt(D))
NEG = -1e30

_CACHED = {}


def _i0_bd(tt):  # first needed i for BD t-tile tt
    return max(0, 384 - tt * 128)


def _i0_j(jt):  # first needed i for score j-tile jt
    return max(0, (jt - 4) * 128)


def _build_nc():
    nc = bacc.Bacc("TRN2", target_bir_lowering=False, debug=False)

    xq = nc.dram_tensor("xq", [1024, MD], F32, kind="ExternalInput")
    hk = nc.dram_tensor("hk", [TOTAL, MD], F32, kind="ExternalInput")
    hv = nc.dram_tensor("hv", [TOTAL, MD], F32, kind="ExternalInput")
    Rr = nc.dram_tensor("Rr", [TOTAL, MD], F32, kind="ExternalInput")
    Wq = nc.dram_tensor("Wq", [MD, H * D], F32, kind="ExternalInput")
    Wkv = nc.dram_tensor("Wkv", [MD, 2 * H * D], F32, kind="ExternalInput")
    Wr = nc.dram_tensor("Wr", [MD, H * D], F32, kind="ExternalInput")
    Wmlp = nc.dram_tensor("Wmlp", [H * D, MD], F32, kind="ExternalInput")
    u1x = nc.dram_tensor("u1x", [128, 1024], F32, kind="ExternalInput")
    u2x = nc.dram_tensor("u2x", [128, 1024], F32, kind="ExternalInput")
    gammab = nc.dram_tensor("gammab", [128, 128], F32, kind="ExternalInput")
    betab = nc.dram_tensor("betab", [128, 128], F32, kind="ExternalInput")
    out = nc.dram_tensor("out", [1024, MD], F32, kind="ExternalOutput")

    with tile.TileContext(nc) as tc:
        _emit(nc, tc, xq, hk, hv, Rr, Wq, Wkv, Wr, Wmlp, u1x, u2x, gammab, betab, out)
    nc.compile()
    return nc


def _emit(nc, tc, xq, hk, hv, Rr, Wq, Wkv, Wr, Wmlp, u1x, u2x, gammab, betab, out):
    from contextlib import ExitStack

    ctx = ExitStack()
    with ctx:
        persist = ctx.enter_context(tc.tile_pool(name="persist", bufs=1))
        big = ctx.enter_context(tc.tile_pool(name="big", bufs=1))
        dram = ctx.enter_context(tc.tile_pool(name="dram", bufs=1, space="DRAM"))

        # ---------- constants ----------
        ident = persist.tile([128, 128], BF16)
        nc.vector.memset(ident[:], 0.0)
        nc.gpsimd.affine_select(
            out=ident[:], in_=ident[:], compare_op=mybir.AluOpType.not_equal,
            fill=1.0, base=0, pattern=[[-1, 128]], channel_multiplier=1,
        )
        ones_col = persist.tile([128, 1], BF16)
        nc.vector.memset(ones_col[:], 1.0)
        ones_row = persist.tile([1, 128], BF16)
        nc.vector.memset(ones_row[:], 1.0)
        eps_t = persist.tile([128, 1], F32)
        nc.vector.memset(eps_t[:], 1e-5)
        zeros_bf = persist.tile([128, 512], BF16)
        nc.vector.memset(zeros_bf[:], 0.0)

        # ---------- load weights / broadcast tensors ----------
        def load_cast(src, cols, nm):
            f = big.tile([128, cols], F32, tag="ldstage", name=f"ld_{nm}")
            nc.sync.dma_start(f[:], src[:])
            b_ = persist.tile([128, cols], BF16, tag=nm, name=nm)
            nc.vector.tensor_copy(b_[:], f[:])
            return b_

        wq_bf = load_cast(Wq, 1024, "wq_bf")
        wkv_bf = load_cast(Wkv, 2048, "wkv_bf")
        wr_bf = load_cast(Wr, 1024, "wr_bf")
        u1x_bf = load_cast(u1x, 1024, "u1x_bf")
        u2x_bf = load_cast(u2x, 1024, "u2x_bf")

        wmlp_f = big.tile([128, 8, 128], F32, tag="ldstage")
        nc.sync.dma_start(wmlp_f[:], Wmlp[:].rearrange("(e p) m -> p e m", p=128))
        wmlp_bf = persist.tile([128, 8, 128], BF16)
        nc.vector.tensor_copy(wmlp_bf[:], wmlp_f[:])

        gam = persist.tile([128, 128], F32)
        nc.sync.dma_start(gam[:], gammab[:])
        bet = persist.tile([128, 128], F32)
        nc.sync.dma_start(bet[:], betab[:])

        # ---------- load + transpose activations ----------
        phaseA = ExitStack()
        tp_ps = phaseA.enter_context(tc.tile_pool(name="tp_ps", bufs=2, space="PSUM"))

        x8_f = persist.tile([128, 8, 128], F32)  # xq rows kept fp32 for residual
        nc.sync.dma_start(x8_f[:], xq[:].rearrange("(t p) c -> p t c", p=128))

        def transpose_in(src_dram, nm, keep_f32=None):
            """[1024,128] dram -> [128,1024] bf16 SBUF (columns = row index)."""
            if keep_f32 is None:
                stage = big.tile([128, 8, 128], F32, tag="ldstage", name=f"st_{nm}")
                nc.sync.dma_start(stage[:], src_dram[:].rearrange("(t p) c -> p t c", p=128))
            else:
                stage = keep_f32
            stage_bf = big.tile([128, 8, 128], BF16, tag="tstage", name=f"sb_{nm}")
            nc.vector.tensor_copy(stage_bf[:], stage[:])
            dst = persist.tile([128, 1024], BF16, tag=nm, name=nm)
            for t in range(8):
                ps = tp_ps.tile([128, 128], BF16, tag="tp")
                nc.tensor.transpose(ps[:], stage_bf[:, t, :], ident[:])
                nc.vector.tensor_copy(dst[:, t * 128:(t + 1) * 128], ps[:])
            return dst

        xqT = transpose_in(xq, "xqT", keep_f32=x8_f)
        hkT = transpose_in(hk, "hkT")
        hvT = transpose_in(hv, "hvT")
        rT_in = transpose_in(Rr, "rT_in")

        # ---------- projections ----------
        pj_ps = phaseA.enter_context(tc.tile_pool(name="pj_ps", bufs=4, space="PSUM"))

        # kvVT then V (so the big kvVT buffer can be freed before kvKT/qfT alloc)
        with tc.tile_pool(name="kvvt_pool", bufs=1) as kvvt_pool:
            kvVT = kvvt_pool.tile([128, 16 * 1024], BF16)  # j-layout: col = t*16 + s
            kvVT_w = kvVT[:].rearrange("p (t s) -> p t s", s=16)
            for s in range(16):
                for n2 in range(2):
                    ps = pj_ps.tile([128, 512], F32, tag="pj")
                    nc.tensor.matmul(ps[:], wkv_bf[:, s * 128:(s + 1) * 128],
                                     hvT[:, n2 * 512:(n2 + 1) * 512], start=True, stop=True)
                    nc.vector.tensor_copy(kvVT_w[:, n2 * 512:(n2 + 1) * 512, s], ps[:])

            v_bf = persist.tile([128, 16 * 8 * 128], BF16)  # [(half,h,jt) tiles of [j,128]]
            for half in range(2):
                for h in range(H):
                    for jt in range(8):
                        base = (half * 512 + h * 64) * 16 + jt * 128
                        ps = tp_ps.tile([128, 128], BF16, tag="tp")
                        nc.tensor.transpose(ps[:], kvVT[:, base:base + 128], ident[:])
                        c0 = ((half * 8 + h) * 8 + jt) * 128
                        nc.vector.tensor_copy(v_bf[:, c0:c0 + 128], ps[:])

        kvKT = persist.tile([128, 16 * 1024], BF16)  # j-layout: col = t*16 + s
        kvKT_w = kvKT[:].rearrange("p (t s) -> p t s", s=16)
        for s in range(16):
            for n2 in range(2):
                ps = pj_ps.tile([128, 512], F32, tag="pj")
                nc.tensor.matmul(ps[:], wkv_bf[:, s * 128:(s + 1) * 128],
                                 hkT[:, n2 * 512:(n2 + 1) * 512], start=True, stop=True)
                nc.scalar.copy(kvKT_w[:, n2 * 512:(n2 + 1) * 512, s], ps[:])

        qfT1 = persist.tile([128, 8 * 1024], BF16)  # j-layout: col = r*8 + e
        qfT2 = persist.tile([128, 8 * 1024], BF16)
        qfT1_w = qfT1[:].rearrange("p (r e) -> p r e", e=8)
        qfT2_w = qfT2[:].rearrange("p (r e) -> p r e", e=8)
        for e in range(8):
            for n2 in range(2):
                ps = pj_ps.tile([128, 512], F32, tag="pj")
                nc.tensor.matmul(ps[:], wq_bf[:, e * 128:(e + 1) * 128],
                                 xqT[:, n2 * 512:(n2 + 1) * 512], start=True, stop=True)
                nc.vector.tensor_add(qfT1_w[:, n2 * 512:(n2 + 1) * 512, e], ps[:],
                                     u1x_bf[:, n2 * 512:(n2 + 1) * 512])
                nc.vector.tensor_add(qfT2_w[:, n2 * 512:(n2 + 1) * 512, e], ps[:],
                                     u2x_bf[:, n2 * 512:(n2 + 1) * 512])

        rfT = persist.tile([128, 8 * 1024], BF16)  # j-layout: col = r*8 + e
        rfT_w = rfT[:].rearrange("p (r e) -> p r e", e=8)
        for e in range(8):
            for n2 in range(2):
                ps = pj_ps.tile([128, 512], F32, tag="pj")
                nc.tensor.matmul(ps[:], wr_bf[:, e * 128:(e + 1) * 128],
                                 rT_in[:, n2 * 512:(n2 + 1) * 512], start=True, stop=True)
                nc.scalar.copy(rfT_w[:, n2 * 512:(n2 + 1) * 512, e], ps[:])

        # BD shift scratch (ping-pong, bf16), rows 1024..1535 zeroed once
        scr = [dram.tile([1536, 512], BF16, tag=f"scr{i}", name=f"scr{i}") for i in range(2)]
        for s_ in scr:
            for k in range(4):
                nc.sync.dma_start(s_[1024 + k * 128:1024 + (k + 1) * 128, :], zeros_bf[:])

        attTall = persist.tile([128, 2 * 8 * 512], BF16)
        phaseA.close()  # release transpose/projection PSUM pools

        # ---------- attention ----------
        at_s = ctx.enter_context(tc.tile_pool(name="at_s", bufs=2, space="PSUM"))
        at_att = ctx.enter_context(tc.tile_pool(name="at_att", bufs=2, space="PSUM"))
        at_den = ctx.enter_context(tc.tile_pool(name="at_den", bufs=1, space="PSUM"))
        at_bc = ctx.enter_context(tc.tile_pool(name="at_bc", bufs=1, space="PSUM"))
        at_bd = ctx.enter_context(tc.tile_pool(name="at_bd", bufs=2, space="PSUM"))
        work = ctx.enter_context(tc.tile_pool(name="work", bufs=3))

        for pair in range(16):
            half, h = divmod(pair, H)
            b = half
            sc = scr[pair % 2]
            base_kv = half * 512 + h * 64
            qj = (b * 512 + h * 64) * 8  # start col of this head in qfT j-layout

            # BD^T tiles -> scratch
            for tt in range(8):
                i0 = _i0_bd(tt)
                n = 512 - i0
                ps = at_bd.tile([128, 512], F32, tag="bd")
                nc.tensor.matmul(
                    ps[:, :n],
                    rfT[:, h * 1024 + tt * 128: h * 1024 + (tt + 1) * 128],
                    qfT2[:, qj + i0: qj + 512],
                    start=True, stop=True,
                )
                bd_sb = work.tile([128, 512], BF16, tag="bdsb")
                if tt % 2 == 0:
                    nc.vector.tensor_copy(bd_sb[:, :n], ps[:, :n])
                else:
                    nc.scalar.copy(bd_sb[:, :n], ps[:, :n])
                nc.sync.dma_start(sc[tt * 128:(tt + 1) * 128, i0:512], bd_sb[:, :n])

            # score^T tiles, exp, denominators, V matmul
            den_ps = at_den.tile([1, 512], F32, tag="den")
            att_ps = at_att.tile([128, 512], F32, tag="att")
            for jt in range(8):
                i0 = _i0_j(jt)
                n = 512 - i0

                bdsT = work.tile([128, 512], BF16, tag="bdsT")
                src = bass.AP(
                    tensor=sc.tensor,
                    offset=sc[:].offset + (jt * 128 + 511 - i0) * 512 + i0,
                    ap=[[512, 128], [1 - 512, n]],
                )
                nc.sync.dma_start(bdsT[:, :n], src)
                if jt >= 4:
                    nc.gpsimd.affine_select(
                        out=bdsT[:, 0:128], in_=bdsT[:, 0:128],
                        compare_op=mybir.AluOpType.is_ge,
                        fill=NEG, base=0, pattern=[[1, 128]], channel_multiplier=-1,
                    )

                s_ps = at_s.tile([128, 512], F32, tag="s")
                nc.tensor.matmul(
                    s_ps[:, :n],
                    kvKT[:, base_kv * 16 + jt * 128: base_kv * 16 + (jt + 1) * 128],
                    qfT1[:, qj + i0: qj + 512],
                    start=True, stop=False,
                )
                nc.tensor.matmul(s_ps[:, :n], ident[:], bdsT[:, :n], start=False, stop=True)

                pT = work.tile([128, 512], BF16, tag="pT")
                nc.scalar.activation(
                    out=pT[:, :n], in_=s_ps[:, :n],
                    func=mybir.ActivationFunctionType.Exp, scale=INV_SQRT_D,
                )

                nc.tensor.matmul(den_ps[0:1, i0:512], ones_col[:], pT[:, :n],
                                 start=(jt == 0), stop=(jt == 7))
                vc0 = ((half * 8 + h) * 8 + jt) * 128
                nc.tensor.matmul(att_ps[:, i0:512], v_bf[:, vc0:vc0 + 128], pT[:, :n],
                                 start=(jt == 0), stop=(jt == 7))

            rden = work.tile([1, 512], F32, tag="rden")
            nc.vector.reciprocal(rden[:], den_ps[:])
            rden_bf = work.tile([1, 512], BF16, tag="rdenb")
            nc.vector.tensor_copy(rden_bf[:], rden[:])
            bc_ps = at_bc.tile([128, 512], F32, tag="bc")
            nc.tensor.matmul(bc_ps[:], ones_row[:], rden_bf[:], start=True, stop=True)
            rb = work.tile([128, 512], F32, tag="rb")
            nc.scalar.copy(rb[:], bc_ps[:])
            a0 = (b * 8 + h) * 512
            nc.vector.tensor_mul(attTall[:, a0:a0 + 512], att_ps[:], rb[:])

        # ---------- output: y = att @ Wmlp + x, LayerNorm ----------
        att_r = attTall[:].rearrange("p (bb s e) -> p bb s e", bb=2, e=8)
        for b in range(2):
            for mt in range(4):
                y_ps = at_s.tile([128, 128], F32, tag="s")
                for e in range(8):
                    nc.tensor.matmul(
                        y_ps[:], att_r[:, b, mt * 128:(mt + 1) * 128, e], wmlp_bf[:, e, :],
                        start=(e == 0), stop=(e == 7),
                    )
                y_sb = work.tile([128, 128], F32, tag="ysb")
                nc.vector.tensor_add(y_sb[:], y_ps[:], x8_f[:, b * 4 + mt, :])

                stats = work.tile([128, 6], F32, tag="st")
                nc.vector.bn_stats(out=stats[:], in_=y_sb[:])
                mv = work.tile([128, 2], F32, tag="mv")
                nc.vector.bn_aggr(out=mv[:], in_=stats[:])
                rstd = work.tile([128, 1], F32, tag="rstd")
                nc.scalar.activation(out=rstd[:], in_=mv[:, 1:2],
                                     func=mybir.ActivationFunctionType.Sqrt,
                                     bias=eps_t[:], scale=1.0)
                nc.vector.reciprocal(rstd[:], rstd[:])
                o_sb = work.tile([128, 128], F32, tag="osb")
                nc.vector.tensor_scalar(
                    out=o_sb[:], in0=y_sb[:], scalar1=mv[:, 0:1], scalar2=rstd[:],
                    op0=mybir.AluOpType.subtract, op1=mybir.AluOpType.mult,
                )
                nc.vector.tensor_mul(o_sb[:], o_sb[:], gam[:])
                nc.vector.tensor_add(o_sb[:], o_sb[:], bet[:])
                nc.sync.dma_start(out[b * 512 + mt * 128: b * 512 + (mt + 1) * 128, :], o_sb[:])


def _make_in_maps(inputs):
    x = np.ascontiguousarray(np.asarray(inputs["x"], dtype=np.float32))
    mem = np.ascontiguousarray(np.asarray(inputs["mem"], dtype=np.float32))
    R = np.ascontiguousarray(np.asarray(inputs["R"], dtype=np.float32))[-TOTAL:]
    u1 = np.asarray(inputs["u1"], dtype=np.float32).reshape(H, D)
    u2 = np.asarray(inputs["u2"], dtype=np.float32).reshape(H, D)
    gamma = np.asarray(inputs["gamma"], dtype=np.float32)
    beta = np.asarray(inputs["beta"], dtype=np.float32)

    u1x = np.zeros((128, 1024), np.float32)
    u2x = np.zeros((128, 1024), np.float32)
    for b2 in range(2):
        for h in range(H):
            u1x[:, b2 * 512 + h * 64: b2 * 512 + (h + 1) * 64] = u1[h][:, None]
            u2x[:, b2 * 512 + h * 64: b2 * 512 + (h + 1) * 64] = u2[h][:, None]
    gammab = np.tile(gamma[None, :], (128, 1)).astype(np.float32)
    betab = np.tile(beta[None, :], (128, 1)).astype(np.float32)
    shared = {
        "Rr": R,
        "Wq": np.ascontiguousarray(np.asarray(inputs["Wq"], np.float32)),
        "Wkv": np.ascontiguousarray(np.asarray(inputs["Wkv"], np.float32)),
        "Wr": np.ascontiguousarray(np.asarray(inputs["Wr"], np.float32)),
        "Wmlp": np.ascontiguousarray(np.asarray(inputs["Wmlp"], np.float32)),
        "u1x": u1x, "u2x": u2x, "gammab": gammab, "betab": betab,
    }
    maps = []
    for c in range(NCORES):
        m = dict(shared)
        m["xq"] = np.ascontiguousarray(x[2 * c:2 * c + 2].reshape(1024, MD))
        m["hk"] = np.ascontiguousarray(np.concatenate([mem[c], x[c]], axis=0))
        m["hv"] = np.ascontiguousarray(np.concatenate([mem[8 + c], x[8 + c]], axis=0))
        maps.append(m)
    return maps


def get_nc():
    if "nc" not in _CACHED:
        _CACHED["nc"] = _build_nc()
    return _CACHED["nc"]


def kernel(**inputs) -> np.ndarray:
    nc = get_nc()
    in_maps = _make_in_maps(inputs)
    res = run_bass_kernel_spmd(nc, in_maps, list(range(NCORES))).results
    full = np.empty((B, SEG, MD), np.float32)
    for c in range(NCORES):
        full[2 * c:2 * c + 2] = res[c]["out"].reshape(2, SEG, MD)
    return full
